# revision 1
# baseline (speedup 1.0000x reference)
"""CornerPool block (conv/BN/cummax-pool residual block) on 8 Trainium2
NeuronCores, pure data-parallel over batch (1 sample per core).

Reference computation per sample (x: [256, 128, 128] f32):
    res    = BN(conv1x1(x, w_res))
    p1     = relu(BN(conv3x3(x, w_vpre)))        # 256 -> 64
    pool1  = reverse-cummax(p1, axis=H)          # TopPool
    p2     = relu(BN(conv3x3(x, w_hpre)))        # 256 -> 64
    pool2  = reverse-cummax(p2, axis=W)          # LeftPool
    merged = BN(conv3x3(pool1 + pool2, w_add))   # 64 -> 256
    out    = relu(res + merged)
    y      = relu(BN(conv3x3(out, w_post)))      # 256 -> 256

Kernel strategy (per core), v2 -- full bf16 datapath:
  * BN folded into conv weights/biases host-side; every conv is a
    sum-of-9-shifted-taps matmul accumulation in PSUM (channels on the
    partition dim, pixels on the free dim, N=512 = 4 image rows).
    All matmuls bf16 x bf16 -> f32 PSUM (216 ns/MM issue rate at N=512,
    vs 227-236 for float32r).
  * x ([2kt x 128, 130, 130] padded) and `out` ([2ct x 128, 130, 130])
    are fully SBUF-resident in bf16: x is DMAed once (no halo re-reads,
    no phase-C re-read), `out` never bounces through DRAM.
  * vpre+hpre convs fused into one matmul stream (64+64 output channels
    fill the 128-wide stationary operand).
  * TopPool as per-strip DVE tensor_max chains (hidden under A matmuls).
    LeftPool as TWO masked tensor_tensor_scan instructions per 32-row
    group (state = (mask * state) max data over the reversed flattened
    row block; the mask zeroes at row boundaries to reset the running
    max) -- one on Vector, one on GpSimd, ~5.5 us/group instead of a
    127-op serial tensor_max chain (~32 us/group).
  * The merged conv contracts over all 128 partitions with the 64-row
    weight block replicated, computing conv(pool1 + pool2) without
    materializing the sum; the res 1x1 conv accumulates into the same
    PSUM group.
  * All pad-region zeroing via engine memsets (the v1 zero-fill DMAs
    with 4-byte elements serialized the Sync queue for ~40 us before
    the first matmul).
"""

import sys

import numpy as np

if "/opt/trn_rl_repo" not in sys.path:
    sys.path.insert(0, "/opt/trn_rl_repo")

EPS = 1e-5
C, M = 256, 64
B, H, W = 8, 128, 128
S = 4                      # output rows per strip
NS = H // S                # 32 strips
HP, WP = H + 2, W + 2      # padded spatial dims
N_CORES = 8
GR = 16                    # rows per leftpool scan (2 scans per 32-row group)

_CACHE = {}


def _patch_tile_drain():
    """This walrus build rejects >2 packed sync waits on the TileContext
    exit Drain. Split them into standalone wait_ge instructions."""
    import concourse.tile as tile
    from concourse.vector_clock import ScopedClock

    if getattr(tile.TileContext._drain_and_barrier, "_split_waits", False):
        return

    def _drain_and_barrier(self, tick_clock, wait_clock):
        nc = self.nc
        probe = nc.sync.nop(nofuse=True)
        wait_clock.add_sem_waits(
            probe.ins, ScopedClock({None: tick_clock.global_clock})
        )
        waits = list(probe.ins.sync_info.on_wait)
        if len(waits) > 1:
            probe.ins.sync_info.on_wait = waits[:1]
            sems_by_id = {s.num: s for s in wait_clock.sems.allocated().values()}
            for w in waits[1:]:
                nc.sync.wait_ge(sems_by_id[w.id], w.wait_value)
        nc.sync.drain()
        nc.all_engine_barrier()
        popped = nc._tile_sem_poison_stack.pop()
        assert popped is self._sem_poison
        nc.clear_and_free_semaphores(list(self.sems.allocated().values()))
        nc.all_engine_barrier()

    _drain_and_barrier._split_waits = True
    tile.TileContext._drain_and_barrier = _drain_and_barrier


TAPS = [(dy, dx) for dy in range(3) for dx in range(3)]


def _legalize_waits(nc, mybir):
    """This walrus build accepts at most ONE sync wait per instruction
    (any class). Split excess waits into single-wait NoOps emitted just
    before the instruction on the same engine sequencer."""
    for f in nc.m.functions:
        for bb in f.blocks:
            insts = bb.instructions
            out = []
            for inst in insts:
                si = inst.sync_info
                waits = list(si.on_wait) if si is not None else []
                if len(waits) > 1:
                    for j, w in enumerate(waits[:-1]):
                        noop = mybir.InstNoOp(
                            name=f"{inst.name}-ws{j}",
                            sync_info=mybir.SyncInfo(on_wait=[w], on_update=[]),
                            bass_nofuse=True,
                            engine=inst.engine,
                        )
                        nc.register_instruction(noop)
                        out.append(noop)
                    si.on_wait = waits[-1:]
                out.append(inst)
            insts[:] = out


def _rev_flat(ap2d, flat_len):
    """Reversed, flattened single-free-dim view of a row-contiguous 2D
    base AP (partition dim preserved)."""
    c = ap2d.copy()
    c.ap[1] = [-1, flat_len]
    c.offset = c.offset + flat_len - 1
    return c


def build_nc():
    import concourse.bass as bass
    import concourse.mybir as mybir
    import concourse.tile as tile

    _patch_tile_drain()
    bf16 = mybir.dt.bfloat16
    f32 = mybir.dt.float32
    Relu = mybir.ActivationFunctionType.Relu
    Mult = mybir.AluOpType.mult
    Max = mybir.AluOpType.max

    nc = bass.Bass()
    x_d = nc.declare_dram_parameter("x_s", [C, HP, WP], bf16, isOutput=False)
    # lhsT weight banks, laid out [k(part), idx, m]
    wvh_d = nc.declare_dram_parameter("w_vh", [128, 18, 128], bf16, isOutput=False)
    wres_d = nc.declare_dram_parameter("w_res_l", [128, 4, 128], bf16, isOutput=False)
    wmrg_d = nc.declare_dram_parameter("w_mrg", [128, 18, 128], bf16, isOutput=False)
    wpost_d = nc.declare_dram_parameter("w_post_l", [128, 36, 128], bf16, isOutput=False)
    bias_d = nc.declare_dram_parameter("biases", [128, 5], f32, isOutput=False)
    mask_d = nc.declare_dram_parameter("mask", [64, GR * WP], bf16, isOutput=False)
    y_d = nc.declare_dram_parameter("y", [C, H, W], f32, isOutput=True)

    with tile.TileContext(nc) as tc:
        with (
            tc.tile_pool(name="const", bufs=1) as constp,
            tc.tile_pool(name="big", bufs=1) as bigp,
            tc.tile_pool(name="stage", bufs=4) as stagep,
            tc.tile_pool(name="psum", bufs=8, space="PSUM") as psump,
        ):
            # ---- persistent buffers ----
            xt = [bigp.tile([128, HP, WP], bf16, name=f"xt{kt}") for kt in range(2)]
            pooled = bigp.tile([128, HP, WP], bf16)
            outt = [bigp.tile([128, HP, WP], bf16, name=f"outt{ct}") for ct in range(2)]

            wvh = constp.tile([128, 18, 128], bf16)
            wres = constp.tile([128, 4, 128], bf16)
            wmrg = constp.tile([128, 18, 128], bf16)
            wpost = constp.tile([128, 36, 128], bf16)
            bias = constp.tile([128, 5], f32)
            mask = constp.tile([128, GR * WP], bf16)

            # ---- pad-region zeroing via memsets (no DMA involved) ----
            for t in (pooled, outt[0], outt[1]):
                nc.vector.memset(t[:, 0, :], 0.0)
                nc.vector.memset(t[:, HP - 1, :], 0.0)
                nc.gpsimd.memset(t[:, 1 : HP - 1, 0:1], 0.0)
                nc.gpsimd.memset(t[:, 1 : HP - 1, WP - 1 : WP], 0.0)

            # ---- constants: small stuff + wvh on the gpsimd queue ----
            nc.gpsimd.dma_start(bias[:], bias_d[:])
            nc.gpsimd.dma_start(mask[64:128, :], mask_d[:])
            for j in range(0, 18, 3):
                nc.gpsimd.dma_start(wvh[:, j : j + 3, :], wvh_d[:, j : j + 3, :])

            # ---- x: resident, DMAed once in descending row chunks ----
            RC = 16
            lims = list(range(HP, 0, -RC)) + [0]
            for hi, lo in zip(lims, lims[1:]):
                for kt in range(2):
                    nc.sync.dma_start(
                        xt[kt][:, lo:hi, :], x_d[kt * 128 : (kt + 1) * 128, lo:hi, :]
                    )

            # ---- deferred consts (needed from phase C on) ----
            def emit_deferred_consts():
                nc.gpsimd.dma_start(wres[:], wres_d[:])
                for j in range(0, 18, 5):
                    e = min(j + 5, 18)
                    nc.gpsimd.dma_start(wmrg[:, j:e, :], wmrg_d[:, j:e, :])
                for j in range(0, 36, 5):
                    e = min(j + 5, 36)
                    nc.gpsimd.dma_start(wpost[:, j:e, :], wpost_d[:, j:e, :])

            def emit_A(s):
                """conv(x, [w_vpre|w_hpre]) + BN + relu for rows 4s..4s+3."""
                r = S * s
                ps = psump.tile([128, S * W], f32, tag="ps")
                i = 0
                for kt in range(2):
                    for t, (dy, dx) in enumerate(TAPS):
                        nc.tensor.matmul(
                            ps[:],
                            wvh[:, kt * 9 + t, :],
                            xt[kt][:, r + dy : r + dy + S, dx : dx + W],
                            start=(i == 0),
                            stop=(i == 17),
                        )
                        i += 1
                nc.scalar.activation(
                    pooled[:, r + 1 : r + 1 + S, 1 : 1 + W],
                    ps[:],
                    Relu,
                    bias=bias[:, 0:1],
                )

            def emit_toppool(s):
                r = S * s
                for y in range(min(H - 2, r + S - 1), r - 1, -1):
                    nc.vector.tensor_max(
                        pooled[0:64, y + 1, 1 : 1 + W],
                        pooled[0:64, y + 1, 1 : 1 + W],
                        pooled[0:64, y + 2, 1 : 1 + W],
                    )

            def emit_leftpool(k):
                """Masked reverse scans over rows 32k+1 .. 32k+32 of p2."""
                for half, eng in ((0, nc.vector), (1, nc.vector)):
                    rlo = 32 * k + 1 + GR * half
                    base = pooled[64:128, rlo, :]
                    eng.tensor_tensor_scan(
                        _rev_flat(base, GR * WP),
                        _rev_flat(mask[64:128, 0:WP], GR * WP),
                        _rev_flat(base, GR * WP),
                        0.0,
                        Mult,
                        Max,
                    )

            def emit_C(s):
                """res conv + merged conv + add + relu -> out[ct] rows."""
                r = S * s
                for ct in range(2):
                    ps = psump.tile([128, S * W], f32, tag="ps")
                    for kt in range(2):
                        nc.tensor.matmul(
                            ps[:],
                            wres[:, ct * 2 + kt, :],
                            xt[kt][:, r + 1 : r + 1 + S, 1 : 1 + W],
                            start=(kt == 0),
                            stop=False,
                        )
                    for t, (dy, dx) in enumerate(TAPS):
                        nc.tensor.matmul(
                            ps[:],
                            wmrg[:, ct * 9 + t, :],
                            pooled[:, r + dy : r + dy + S, dx : dx + W],
                            start=False,
                            stop=(t == 8),
                        )
                    nc.scalar.activation(
                        outt[ct][:, r + 1 : r + 1 + S, 1 : 1 + W],
                        ps[:],
                        Relu,
                        bias=bias[:, 1 + ct : 2 + ct],
                    )

            def emit_D(s):
                """post conv + BN + relu -> y strip."""
                r = S * s
                for co in range(2):
                    ps = psump.tile([128, S * W], f32, tag="ps")
                    i = 0
                    for kt in range(2):
                        for t, (dy, dx) in enumerate(TAPS):
                            nc.tensor.matmul(
                                ps[:],
                                wpost[:, co * 18 + kt * 9 + t, :],
                                outt[kt][:, r + dy : r + dy + S, dx : dx + W],
                                start=(i == 0),
                                stop=(i == 17),
                            )
                            i += 1
                    st = stagep.tile([128, S * W], f32, tag="std")
                    nc.scalar.activation(st[:], ps[:], Relu, bias=bias[:, 3 + co : 4 + co])
                    nc.sync.dma_start(y_d[co * 128 : (co + 1) * 128, r : r + S, :], st[:])

            # Software-pipelined wavefront in groups of 8 strips, bottom-up
            # so the reverse-cummax chains unlock consumers early. The C/D
            # batch for group k is emitted AFTER group k-1's conv-A strips:
            # the PE chews a full group of conv-A matmuls while the pool
            # engines run the LeftPool scans the C batch waits for.
            def emit_group_A(k):
                for s in range(8 * k + 7, 8 * k - 1, -1):
                    emit_A(s)
                    emit_toppool(s)
                emit_leftpool(k)

            def emit_group_CD(k):
                c_lo = 8 * k + 1 if k > 0 else 0
                for s in range(min(NS - 1, 8 * k + 8), c_lo - 1, -1):
                    emit_C(s)
                d_hi = NS - 1 if k == 3 else 8 * k + 9
                d_lo = 8 * k + 2 if k > 0 else 0
                for s in range(d_hi, d_lo - 1, -1):
                    emit_D(s)

            emit_deferred_consts()
            emit_group_A(3)
            for k in range(3, -1, -1):
                if k > 0:
                    emit_group_A(k - 1)
                emit_group_CD(k)

    _legalize_waits(nc, mybir)
    return nc


def _fold_bn(w, bn):
    """BN(conv(x, w)) == conv(x, w * s[co]) + t[co]."""
    g, b, m, v = bn[0], bn[1], bn[2], bn[3]
    s = g / np.sqrt(v + EPS)
    t = b - m * s
    return w * s[:, None, None, None], t


def _prep_inputs(x, w_res, bn_res, w_vpre, bn_vpre, w_hpre, bn_hpre,
                 w_add, bn_add, w_post, bn_post):
    import ml_dtypes

    bf16 = ml_dtypes.bfloat16
    x = np.asarray(x, np.float32)
    xp = np.zeros((B, C, HP, WP), bf16)
    xp[:, :, 1 : 1 + H, 1 : 1 + W] = x.astype(bf16)
    x = xp
    w_res_s, t_res = _fold_bn(np.asarray(w_res, np.float32), np.asarray(bn_res, np.float32))
    w_vpre_s, t_vpre = _fold_bn(np.asarray(w_vpre, np.float32), np.asarray(bn_vpre, np.float32))
    w_hpre_s, t_hpre = _fold_bn(np.asarray(w_hpre, np.float32), np.asarray(bn_hpre, np.float32))
    w_add_s, t_add = _fold_bn(np.asarray(w_add, np.float32), np.asarray(bn_add, np.float32))
    w_post_s, t_post = _fold_bn(np.asarray(w_post, np.float32), np.asarray(bn_post, np.float32))

    # w_vh[k, kt*9+t, m]: m<64 vpre, m>=64 hpre; lhsT[k, m] = w[m, kt*128+k, dy, dx]
    w_vh = np.zeros((128, 18, 128), np.float32)
    for kt in range(2):
        for t, (dy, dx) in enumerate(TAPS):
            blk = kt * 128
            w_vh[:, kt * 9 + t, 0:64] = w_vpre_s[:, blk : blk + 128, dy, dx].T
            w_vh[:, kt * 9 + t, 64:128] = w_hpre_s[:, blk : blk + 128, dy, dx].T

    # w_res_l[k, ct*2+kt, m] = w_res_s[ct*128+m, kt*128+k]
    w_res_l = np.zeros((128, 4, 128), np.float32)
    for ct in range(2):
        for kt in range(2):
            w_res_l[:, ct * 2 + kt, :] = w_res_s[
                ct * 128 : (ct + 1) * 128, kt * 128 : (kt + 1) * 128, 0, 0
            ].T

    # w_mrg[k, ct*9+t, m] = w_add_s[ct*128+m, k%64, dy, dx]  (row-replicated)
    w_mrg = np.zeros((128, 18, 128), np.float32)
    for ct in range(2):
        for t, (dy, dx) in enumerate(TAPS):
            blkT = w_add_s[ct * 128 : (ct + 1) * 128, :, dy, dx].T  # [64, 128]
            w_mrg[0:64, ct * 9 + t, :] = blkT
            w_mrg[64:128, ct * 9 + t, :] = blkT

    # w_post_l[k, co*18+kt*9+t, m] = w_post_s[co*128+m, kt*128+k, dy, dx]
    w_post_l = np.zeros((128, 36, 128), np.float32)
    for co in range(2):
        for kt in range(2):
            for t, (dy, dx) in enumerate(TAPS):
                w_post_l[:, co * 18 + kt * 9 + t, :] = w_post_s[
                    co * 128 : (co + 1) * 128, kt * 128 : (kt + 1) * 128, dy, dx
                ].T

    biases = np.zeros((128, 5), np.float32)
    biases[0:64, 0] = t_vpre
    biases[64:128, 0] = t_hpre
    t_mrg = t_res + t_add
    biases[:, 1] = t_mrg[0:128]
    biases[:, 2] = t_mrg[128:256]
    biases[:, 3] = t_post[0:128]
    biases[:, 4] = t_post[128:256]

    # LeftPool scan mask: 0 at the two pad columns of every row, 1 inside.
    mask = np.ones((64, GR, WP), np.float32)
    mask[:, :, 0] = 0.0
    mask[:, :, WP - 1] = 0.0

    shared = {
        "w_vh": w_vh.astype(bf16),
        "w_res_l": w_res_l.astype(bf16),
        "w_mrg": w_mrg.astype(bf16),
        "w_post_l": w_post_l.astype(bf16),
        "biases": biases,
        "mask": mask.reshape(64, GR * WP).astype(bf16),
    }
    return x, shared


def kernel(x, w_res, bn_res, w_vpre, bn_vpre, w_hpre, bn_hpre,
           w_add, bn_add, w_post, bn_post):
    from concourse.bass_utils import run_bass_kernel_spmd

    x, shared = _prep_inputs(x, w_res, bn_res, w_vpre, bn_vpre, w_hpre,
                             bn_hpre, w_add, bn_add, w_post, bn_post)

    if "nc" not in _CACHE:
        _CACHE["nc"] = build_nc()
    nc = _CACHE["nc"]

    in_maps = [dict(shared, x_s=np.ascontiguousarray(x[i])) for i in range(N_CORES)]
    res = run_bass_kernel_spmd(nc, in_maps, list(range(N_CORES)))
    return np.stack([res.results[i]["y"] for i in range(N_CORES)]).astype(np.float32)



# revision 14
# speedup vs baseline: 1.2028x; 1.2028x over previous
"""CornerPool block (conv/BN/cummax-pool residual block) on 8 Trainium2
NeuronCores, pure data-parallel over batch (1 sample per core).

Reference computation per sample (x: [256, 128, 128] f32):
    res    = BN(conv1x1(x, w_res))
    p1     = relu(BN(conv3x3(x, w_vpre)))        # 256 -> 64
    pool1  = reverse-cummax(p1, axis=H)          # TopPool
    p2     = relu(BN(conv3x3(x, w_hpre)))        # 256 -> 64
    pool2  = reverse-cummax(p2, axis=W)          # LeftPool
    merged = BN(conv3x3(pool1 + pool2, w_add))   # 64 -> 256
    out    = relu(res + merged)
    y      = relu(BN(conv3x3(out, w_post)))      # 256 -> 256

Kernel strategy (per core), v3 -- 1D Winograd F(2,3) along H:
  * All 3x3 convs run as Winograd F(2,3) in the H direction (dx taps
    stay direct): per 8-row group, 4 position planes M_p accumulate in
    PSUM (6 matmuls each for 256-ch contractions), and the outputs are
    even rows = M0+M1+M2, odd rows = M1-M2-M3 (DVE combines + scalar
    activation for bias+relu). 2/3 the matmul work of direct conv; the
    PE was ~94% busy at the bf16 roofline in v2.
  * The res 1x1 conv folds into the merge conv's M0/M3 PSUM banks for
    free: even-parity res taps accumulate into M0, negated odd-parity
    taps into M3 (o = M1-M2-M3' with M3' = M3 - res_odd).
  * V input planes (V0..V3 row combos) are built by vector tensor ops
    (gpsimd has no TensorTensor opcode on this core version).
  * pooled and out are 4-slot (32-row) ring buffers instead of full
    images -- consumers only need a +-1 group window -- freeing ~78 KB
    of SBUF for the V planes.  One boundary row per 32-row wrap is
    copied aside (bnd) before its slot is overwritten.
  * TopPool as per-row tensor_max chains; LeftPool as masked
    tensor_tensor_scan over 16-row blocks (both vector, as in v2).
  * fp8 was measured (numpy sim) and rejected: a single e4m3 conv costs
    ~4e-2 rel err vs the 2e-2 gate. Winograd-bf16 sims at 5.5e-3.
"""

import sys

import numpy as np

if "/opt/trn_rl_repo" not in sys.path:
    sys.path.insert(0, "/opt/trn_rl_repo")

EPS = 1e-5
C, M = 256, 64
B, H, W = 8, 128, 128
HP, WP = H + 2, W + 2       # padded spatial dims
N_CORES = 8
GR = 16                     # rows per leftpool scan
NG = 16                     # 8-row winograd groups
RNG = 32                    # ring rows (4 slots x 8 rows)

_CACHE = {}


def _patch_tile_drain():
    """This walrus build rejects >2 packed sync waits on the TileContext
    exit Drain. Split them into standalone wait_ge instructions."""
    import concourse.tile as tile
    from concourse.vector_clock import ScopedClock

    if getattr(tile.TileContext._drain_and_barrier, "_split_waits", False):
        return

    def _drain_and_barrier(self, tick_clock, wait_clock):
        nc = self.nc
        probe = nc.sync.nop(nofuse=True)
        wait_clock.add_sem_waits(
            probe.ins, ScopedClock({None: tick_clock.global_clock})
        )
        waits = list(probe.ins.sync_info.on_wait)
        if len(waits) > 1:
            probe.ins.sync_info.on_wait = waits[:1]
            sems_by_id = {s.num: s for s in wait_clock.sems.allocated().values()}
            for w in waits[1:]:
                nc.sync.wait_ge(sems_by_id[w.id], w.wait_value)
        nc.sync.drain()
        nc.all_engine_barrier()
        popped = nc._tile_sem_poison_stack.pop()
        assert popped is self._sem_poison
        nc.clear_and_free_semaphores(list(self.sems.allocated().values()))
        nc.all_engine_barrier()

    _drain_and_barrier._split_waits = True
    tile.TileContext._drain_and_barrier = _drain_and_barrier


def _legalize_waits(nc, mybir):
    """This walrus build accepts at most ONE sync wait per instruction
    (any class). Split excess waits into single-wait NoOps emitted just
    before the instruction on the same engine sequencer."""
    for f in nc.m.functions:
        for bb in f.blocks:
            insts = bb.instructions
            out = []
            for inst in insts:
                si = inst.sync_info
                waits = list(si.on_wait) if si is not None else []
                if len(waits) > 1:
                    for j, w in enumerate(waits[:-1]):
                        noop = mybir.InstNoOp(
                            name=f"{inst.name}-ws{j}",
                            sync_info=mybir.SyncInfo(on_wait=[w], on_update=[]),
                            bass_nofuse=True,
                            engine=inst.engine,
                        )
                        nc.register_instruction(noop)
                        out.append(noop)
                    si.on_wait = waits[-1:]
                out.append(inst)
            insts[:] = out


def _rev_flat(ap2d, flat_len):
    """Reversed, flattened single-free-dim view of a row-contiguous 2D
    base AP (partition dim preserved)."""
    c = ap2d.copy()
    c.ap[1] = [-1, flat_len]
    c.offset = c.offset + flat_len - 1
    return c


def build_nc():
    import concourse.bass as bass
    import concourse.mybir as mybir
    import concourse.tile as tile

    _patch_tile_drain()
    bf16 = mybir.dt.bfloat16
    f32 = mybir.dt.float32
    Relu = mybir.ActivationFunctionType.Relu
    Mult = mybir.AluOpType.mult
    Max = mybir.AluOpType.max

    nc = bass.Bass()
    x_d = nc.declare_dram_parameter("x_s", [C, HP, WP], bf16, isOutput=False)
    # lhsT weight banks, laid out [k(part), idx, m]
    wA_d = nc.declare_dram_parameter("w_a", [128, 24, 128], bf16, isOutput=False)
    wres_d = nc.declare_dram_parameter("w_res_l", [128, 8, 128], bf16, isOutput=False)
    wmrg_d = nc.declare_dram_parameter("w_mrg", [128, 24, 128], bf16, isOutput=False)
    wpost_d = nc.declare_dram_parameter("w_post_l", [128, 48, 128], bf16, isOutput=False)
    bias_d = nc.declare_dram_parameter("biases", [128, 5], f32, isOutput=False)
    mask_d = nc.declare_dram_parameter("mask", [64, GR * WP], bf16, isOutput=False)
    y_d = nc.declare_dram_parameter("y", [C, H, W], f32, isOutput=True)

    with tile.TileContext(nc) as tc:
        with (
            tc.tile_pool(name="const", bufs=1) as constp,
            tc.tile_pool(name="big", bufs=1) as bigp,
            tc.tile_pool(name="vpl", bufs=2) as vp,
            tc.tile_pool(name="stage", bufs=8) as stagep,
            tc.tile_pool(name="psum", bufs=8, space="PSUM") as psump,
        ):
            # ---- persistent buffers ----
            xt = [bigp.tile([128, HP, WP], bf16, name=f"xt{kt}") for kt in range(2)]
            pring = bigp.tile([128, RNG, WP], bf16)
            oring = [bigp.tile([128, RNG, WP], bf16, name=f"oring{ct}") for ct in range(2)]

            wA = constp.tile([128, 24, 128], bf16)
            wres = constp.tile([128, 8, 128], bf16)
            wmrg = constp.tile([128, 24, 128], bf16)
            wpost = constp.tile([128, 48, 128], bf16)
            bias = constp.tile([128, 5], f32)
            mask = constp.tile([128, GR * WP], bf16)
            zrow = constp.tile([128, 1, WP], bf16)
            bnd = constp.tile([128, 1, WP], bf16)

            # ---- pad-column zeroing + zero row (engine memsets) ----
            nc.vector.memset(zrow[:], 0.0)
            for t in (pring, oring[0], oring[1]):
                nc.vector.memset(t[:, :, 0:1], 0.0)
                nc.gpsimd.memset(t[:, :, WP - 1 : WP], 0.0)

            # ---- constants: bias/mask/wA up front on the gpsimd queue ----
            nc.gpsimd.dma_start(bias[:], bias_d[:])
            nc.gpsimd.dma_start(mask[64:128, :], mask_d[:])
            for j in range(0, 24, 3):
                nc.gpsimd.dma_start(wA[:, j : j + 3, :], wA_d[:, j : j + 3, :])

            # ---- x: resident, DMAed once in descending row chunks ----
            RC = 16
            lims = list(range(HP, 0, -RC)) + [0]
            for hi, lo in zip(lims, lims[1:]):
                for kt in range(2):
                    nc.sync.dma_start(
                        xt[kt][:, lo:hi, :], x_d[kt * 128 : (kt + 1) * 128, lo:hi, :]
                    )

            def emit_deferred_consts():
                nc.gpsimd.dma_start(wres[:], wres_d[:])
                for j in range(0, 24, 5):
                    e = min(j + 5, 24)
                    nc.gpsimd.dma_start(wmrg[:, j:e, :], wmrg_d[:, j:e, :])
                for j in range(0, 48, 5):
                    e = min(j + 5, 48)
                    nc.gpsimd.dma_start(wpost[:, j:e, :], wpost_d[:, j:e, :])

            # ================= winograd building blocks =================
            # V plane p for a group lives at v[:, 4p:4p+4, :].
            # Image-row formulas (tile t, base row b = 8g):
            #   V0[t] = im(b+2t-1) - im(b+2t+1)
            #   V1[t] = im(b+2t)   + im(b+2t+1)
            #   V2[t] = im(b+2t+1) - im(b+2t)
            #   V3[t] = im(b+2t)   - im(b+2t+2)

            def build_Vx(g):
                """x transforms (vector; gpsimd lacks TensorTensor); x is
                fully padded so image row i lives at padded row i+1 and no
                edge cases exist."""
                vx = [vp.tile([128, 16, WP], bf16, tag=f"vx{kt}", name=f"vx{kt}")
                      for kt in range(2)]
                r = 8 * g  # padded base: im(b+k) = pd[r+k+1]
                for kt in range(2):
                    s, v = xt[kt], vx[kt]
                    nc.vector.tensor_sub(v[:, 0:4, :], s[:, r : r + 7 : 2, :],
                                         s[:, r + 2 : r + 9 : 2, :])
                    nc.vector.tensor_add(v[:, 4:8, :], s[:, r + 1 : r + 8 : 2, :],
                                         s[:, r + 2 : r + 9 : 2, :])
                    nc.vector.tensor_sub(v[:, 8:12, :], s[:, r + 2 : r + 9 : 2, :],
                                         s[:, r + 1 : r + 8 : 2, :])
                    nc.vector.tensor_sub(v[:, 12:16, :], s[:, r + 1 : r + 8 : 2, :],
                                         s[:, r + 3 : r + 10 : 2, :])
                return vx

            def build_Vring(g, ring, tag, top_src, bot_src):
                """Ring transforms on vector. ring rows = image row % 32.
                top_src: AP for image row 8g+8 (ring row, bnd, or zrow).
                bot_src: AP for image row 8g-1 (ring row or zrow)."""
                v = vp.tile([128, 16, WP], bf16, tag=tag, name=tag)
                r = (8 * g) % RNG
                nc.vector.tensor_sub(v[:, 0:1, :], bot_src, ring[:, r + 1 : r + 2, :])
                nc.vector.tensor_sub(v[:, 1:4, :], ring[:, r + 1 : r + 6 : 2, :],
                                     ring[:, r + 3 : r + 8 : 2, :])
                nc.vector.tensor_add(v[:, 4:8, :], ring[:, r : r + 7 : 2, :],
                                     ring[:, r + 1 : r + 8 : 2, :])
                nc.vector.tensor_sub(v[:, 8:12, :], ring[:, r + 1 : r + 8 : 2, :],
                                     ring[:, r : r + 7 : 2, :])
                nc.vector.tensor_sub(v[:, 12:15, :], ring[:, r : r + 5 : 2, :],
                                     ring[:, r + 2 : r + 7 : 2, :])
                nc.vector.tensor_sub(v[:, 15:16, :], ring[:, r + 6 : r + 7, :], top_src)
                return v

            def combine_act(ps, dst_e, dst_o, bias_col):
                """e = M0+M1+M2, o = M1-M2-M3, then bias+relu into the
                strided dsts. Ops may read at most ONE PSUM operand
                (NCC_IBVF027), so the shared M1/M2 are staged to SBUF as
                bf16 by scalar-engine copies; DVE does the bf16 partial
                combine plus one single-PSUM finish per parity."""
                c1 = stagep.tile([128, 512], bf16, tag="cp", name="c1")
                nc.scalar.copy(c1, ps[1][:])
                c2 = stagep.tile([128, 512], bf16, tag="cp", name="c2")
                nc.scalar.copy(c2, ps[2][:])
                e0 = stagep.tile([128, 512], bf16, tag="eo16", name="e0")
                nc.vector.tensor_add(e0, c1, c2)
                e = stagep.tile([128, 512], f32, tag="eo", name="e")
                nc.vector.tensor_add(e, e0, ps[0][:])
                nc.scalar.activation(dst_e, e, Relu, bias=bias[:, bias_col : bias_col + 1])
                o0 = stagep.tile([128, 512], bf16, tag="eo16", name="o0")
                nc.vector.tensor_sub(o0, c1, c2)
                o = stagep.tile([128, 512], f32, tag="eo", name="o")
                nc.vector.tensor_sub(o, o0, ps[3][:])
                nc.scalar.activation(dst_o, o, Relu, bias=bias[:, bias_col : bias_col + 1])

            def emit_A(g, vx):
                """winograd vpre|hpre conv + BN + relu -> pring rows."""
                ps = [psump.tile([128, 512], f32, tag="ps", name="ps") for _ in range(4)]
                for p in range(4):
                    i = 0
                    for kt in range(2):
                        for dx in range(3):
                            nc.tensor.matmul(
                                ps[p][:],
                                wA[:, kt * 12 + p * 3 + dx, :],
                                vx[kt][:, 4 * p : 4 * p + 4, dx : dx + W],
                                start=(i == 0),
                                stop=(i == 5),
                            )
                            i += 1
                r = (8 * g) % RNG
                combine_act(ps,
                            pring[:, r : r + 7 : 2, 1 : 1 + W],
                            pring[:, r + 1 : r + 8 : 2, 1 : 1 + W], 0)

            def emit_tp(g):
                """TopPool chain rows 8g+7..8g (vector). Row y reads the
                already-final row y+1 (ring row (y+1)%32)."""
                for y in range(min(H - 2, 8 * g + 7), 8 * g - 1, -1):
                    nc.vector.tensor_max(
                        pring[0:64, y % RNG, :],
                        pring[0:64, y % RNG, :],
                        pring[0:64, (y + 1) % RNG, :],
                    )

            def emit_lp(h):
                """Masked reverse scan over ring rows 16h..16h+15 (vector)."""
                base = pring[64:128, 16 * h, :]
                nc.vector.tensor_tensor_scan(
                    _rev_flat(base, GR * WP),
                    _rev_flat(mask[64:128, 0:WP], GR * WP),
                    _rev_flat(base, GR * WP),
                    0.0,
                    Mult,
                    Max,
                )

            def emit_C(g, vpl):
                """winograd merge conv (+folded res conv) + relu -> oring."""
                r = (8 * g) % RNG
                for ct in range(2):
                    ps = [psump.tile([128, 512], f32, tag="ps", name="ps")
                          for _ in range(4)]
                    for p in range(4):
                        for dx in range(3):
                            nc.tensor.matmul(
                                ps[p][:],
                                wmrg[:, ct * 12 + p * 3 + dx, :],
                                vpl[:, 4 * p : 4 * p + 4, dx : dx + W],
                                start=(dx == 0),
                                stop=(dx == 2 and p in (1, 2)),
                            )
                    for kt in range(2):  # res even -> M0
                        nc.tensor.matmul(
                            ps[0][:],
                            wres[:, ct * 2 + kt, :],
                            xt[kt][:, 8 * g + 1 : 8 * g + 8 : 2, 1 : 1 + W],
                            start=False,
                            stop=(kt == 1),
                        )
                    for kt in range(2):  # -res odd -> M3
                        nc.tensor.matmul(
                            ps[3][:],
                            wres[:, 4 + ct * 2 + kt, :],
                            xt[kt][:, 8 * g + 2 : 8 * g + 9 : 2, 1 : 1 + W],
                            start=False,
                            stop=(kt == 1),
                        )
                    combine_act(ps,
                                oring[ct][:, r : r + 7 : 2, 1 : 1 + W],
                                oring[ct][:, r + 1 : r + 8 : 2, 1 : 1 + W], 1 + ct)

            def emit_D(g, vo):
                """winograd post conv + BN + relu -> y strip via DMA."""
                for co in range(2):
                    ps = [psump.tile([128, 512], f32, tag="ps", name="ps")
                          for _ in range(4)]
                    for p in range(4):
                        i = 0
                        for kt in range(2):
                            for dx in range(3):
                                nc.tensor.matmul(
                                    ps[p][:],
                                    wpost[:, co * 24 + kt * 12 + p * 3 + dx, :],
                                    vo[kt][:, 4 * p : 4 * p + 4, dx : dx + W],
                                    start=(i == 0),
                                    stop=(i == 5),
                                )
                                i += 1
                    se = stagep.tile([128, 512], f32, tag="eo", name="se")
                    so = stagep.tile([128, 512], f32, tag="eo", name="so")
                    combine_act(ps, se, so, 3 + co)
                    nc.sync.dma_start(
                        y_d[co * 128 : (co + 1) * 128, 8 * g : 8 * g + 7 : 2, :], se)
                    nc.sync.dma_start(
                        y_d[co * 128 : (co + 1) * 128, 8 * g + 1 : 8 * g + 8 : 2, :], so)

            # ring-row helpers for V-plane boundary sources
            def prow(y):
                return pring[:, y % RNG : y % RNG + 1, :]

            def orow(ct, y):
                return oring[ct][:, y % RNG : y % RNG + 1, :]

            def build_Vp(g):
                top = zrow[:] if g == NG - 1 else (bnd[:] if g % 4 == 3 else prow(8 * g + 8))
                bot = zrow[:] if g == 0 else prow(8 * g - 1)
                return build_Vring(g, pring, "vp", top, bot)

            def build_Vo(g):
                vo = []
                for ct in range(2):
                    top = zrow[:] if g == NG - 1 else orow(ct, 8 * g + 8)
                    bot = zrow[:] if g == 0 else orow(ct, 8 * g - 1)
                    vo.append(build_Vring(g, oring[ct], f"vo{ct}", top, bot))
                return vo

            # ======================= schedule =======================
            # Groups descend (TopPool needs bottom-up). Per 32-row wrap j
            # (groups 4j+3..4j), the deferred C(4j+4)/D(4j+5) of the wrap
            # above run mid-wrap -- after lp(j,1), before A(4j+1) destroys
            # the ring slots they read. Vp(4j+3) is prebuilt mid-wrap
            # (needs bnd + lp(j,1)) so C(4j+3) can start right after A(4j).
            emit_deferred_consts()

            def emit_Ablock(g):
                vx = build_Vx(g)
                emit_A(g, vx)
                emit_tp(g)

            vp_pending = {}   # g -> prebuilt V_pooled tile

            # ---- wrap 3 A phase ----
            emit_Ablock(15)
            emit_Ablock(14)
            emit_lp(1)
            vp_pending[15] = build_Vp(15)     # top row is zrow, no bnd
            emit_Ablock(13)
            emit_Ablock(12)
            emit_lp(0)
            # ---- wrap 3 C/D (no deferred groups yet; D(15) after C(14)) ----
            emit_C(15, vp_pending.pop(15))
            emit_C(14, build_Vp(14))
            emit_D(15, build_Vo(15))
            emit_C(13, build_Vp(13))
            emit_D(14, build_Vo(14))
            for j in (2, 1, 0):
                emit_Ablock(4 * j + 3)
                emit_Ablock(4 * j + 2)
                emit_lp(1)
                # boundary row 32j+32 (ring row 0, wrap j+1 content) -- the
                # only cross-wrap pooled row C(4j+3) needs after A(4j)
                # overwrites slot 0.
                nc.vector.tensor_copy(bnd[:, 0, :], pring[:, 0, :])
                emit_C(4 * j + 4, build_Vp(4 * j + 4))
                emit_D(4 * j + 5, build_Vo(4 * j + 5))
                vp_pending[4 * j + 3] = build_Vp(4 * j + 3)
                emit_Ablock(4 * j + 1)
                emit_Ablock(4 * j + 0)
                emit_lp(0)
                emit_C(4 * j + 3, vp_pending.pop(4 * j + 3))
                emit_D(4 * j + 4, build_Vo(4 * j + 4))
                emit_C(4 * j + 2, build_Vp(4 * j + 2))
                emit_D(4 * j + 3, build_Vo(4 * j + 3))
                emit_C(4 * j + 1, build_Vp(4 * j + 1))
                emit_D(4 * j + 2, build_Vo(4 * j + 2))
            # ---- tail ----
            emit_C(0, build_Vp(0))
            emit_D(1, build_Vo(1))
            emit_D(0, build_Vo(0))

    _legalize_waits(nc, mybir)
    return nc


def _fold_bn(w, bn):
    """BN(conv(x, w)) == conv(x, w * s[co]) + t[co]."""
    g, b, m, v = bn[0], bn[1], bn[2], bn[3]
    s = g / np.sqrt(v + EPS)
    return w * s[:, None, None, None], b - m * s


_G = np.array([[1, 0, 0], [0.5, 0.5, 0.5], [0.5, -0.5, 0.5], [0, 0, 1]], np.float32)


def _prep_inputs(x, w_res, bn_res, w_vpre, bn_vpre, w_hpre, bn_hpre,
                 w_add, bn_add, w_post, bn_post):
    import ml_dtypes

    bf16 = ml_dtypes.bfloat16
    x = np.asarray(x, np.float32)
    xp = np.zeros((B, C, HP, WP), bf16)
    xp[:, :, 1 : 1 + H, 1 : 1 + W] = x.astype(bf16)
    x = xp
    w_res_s, t_res = _fold_bn(np.asarray(w_res, np.float32), np.asarray(bn_res, np.float32))
    w_vpre_s, t_vpre = _fold_bn(np.asarray(w_vpre, np.float32), np.asarray(bn_vpre, np.float32))
    w_hpre_s, t_hpre = _fold_bn(np.asarray(w_hpre, np.float32), np.asarray(bn_hpre, np.float32))
    w_add_s, t_add = _fold_bn(np.asarray(w_add, np.float32), np.asarray(bn_add, np.float32))
    w_post_s, t_post = _fold_bn(np.asarray(w_post, np.float32), np.asarray(bn_post, np.float32))

    # G-transform along dy: U[p, dx] = sum_dy G[p, dy] * w[..., dy, dx]
    def gt(w):  # w: [co, ci, 3, 3] -> [4, co, ci, 3]
        return np.einsum("pd,oidx->poix", _G, w)

    U_v, U_h = gt(w_vpre_s), gt(w_hpre_s)
    U_a, U_p = gt(w_add_s), gt(w_post_s)

    # wA[k, kt*12 + p*3 + dx, m]: m<64 vpre, m>=64 hpre
    w_a = np.zeros((128, 24, 128), np.float32)
    for kt in range(2):
        ks = slice(kt * 128, (kt + 1) * 128)
        for p in range(4):
            for dx in range(3):
                w_a[:, kt * 12 + p * 3 + dx, 0:64] = U_v[p, :, ks, dx].T
                w_a[:, kt * 12 + p * 3 + dx, 64:128] = U_h[p, :, ks, dx].T

    # wres_l[k, par*4 + ct*2 + kt, m] = +-w_res_s[ct*128+m, kt*128+k]
    w_res_l = np.zeros((128, 8, 128), np.float32)
    for ct in range(2):
        for kt in range(2):
            blk = w_res_s[ct * 128 : (ct + 1) * 128,
                          kt * 128 : (kt + 1) * 128, 0, 0].T
            w_res_l[:, ct * 2 + kt, :] = blk
            w_res_l[:, 4 + ct * 2 + kt, :] = -blk

    # wmrg[k, ct*12 + p*3 + dx, m] row-replicated (64-ch contraction)
    w_mrg = np.zeros((128, 24, 128), np.float32)
    for ct in range(2):
        for p in range(4):
            for dx in range(3):
                blkT = U_a[p, ct * 128 : (ct + 1) * 128, :, dx].T  # [64, 128]
                w_mrg[0:64, ct * 12 + p * 3 + dx, :] = blkT
                w_mrg[64:128, ct * 12 + p * 3 + dx, :] = blkT

    # wpost[k, co*24 + kt*12 + p*3 + dx, m]
    w_post_l = np.zeros((128, 48, 128), np.float32)
    for co in range(2):
        for kt in range(2):
            for p in range(4):
                for dx in range(3):
                    w_post_l[:, co * 24 + kt * 12 + p * 3 + dx, :] = U_p[
                        p, co * 128 : (co + 1) * 128,
                        kt * 128 : (kt + 1) * 128, dx].T

    biases = np.zeros((128, 5), np.float32)
    biases[0:64, 0] = t_vpre
    biases[64:128, 0] = t_hpre
    t_mrg = t_res + t_add
    biases[:, 1] = t_mrg[0:128]
    biases[:, 2] = t_mrg[128:256]
    biases[:, 3] = t_post[0:128]
    biases[:, 4] = t_post[128:256]

    # LeftPool scan mask: 0 at the two pad columns of every row, 1 inside.
    mask = np.ones((64, GR, WP), np.float32)
    mask[:, :, 0] = 0.0
    mask[:, :, WP - 1] = 0.0

    shared = {
        "w_a": w_a.astype(bf16),
        "w_res_l": w_res_l.astype(bf16),
        "w_mrg": w_mrg.astype(bf16),
        "w_post_l": w_post_l.astype(bf16),
        "biases": biases,
        "mask": mask.reshape(64, GR * WP).astype(bf16),
    }
    return x, shared


def kernel(x, w_res, bn_res, w_vpre, bn_vpre, w_hpre, bn_hpre,
           w_add, bn_add, w_post, bn_post):
    from concourse.bass_utils import run_bass_kernel_spmd

    x, shared = _prep_inputs(x, w_res, bn_res, w_vpre, bn_vpre, w_hpre,
                             bn_hpre, w_add, bn_add, w_post, bn_post)

    if "nc" not in _CACHE:
        _CACHE["nc"] = build_nc()
    nc = _CACHE["nc"]

    in_maps = [dict(shared, x_s=np.ascontiguousarray(x[i])) for i in range(N_CORES)]
    res = run_bass_kernel_spmd(nc, in_maps, list(range(N_CORES)))
    return np.stack([res.results[i]["y"] for i in range(N_CORES)]).astype(np.float32)
